# revision 26
# baseline (speedup 1.0000x reference)
"""BERT encoder block on 8 Trainium2 NeuronCores.

Strategy: pure data parallelism — batch 8 is split one batch element per core
(no collectives). Each core runs the full encoder block on its [2048, 1024]
slice. All six large matmuls run in fp8e4m3 with DoubleRow perf mode
(K=256 contracted per instruction, ~1.7x TensorE throughput); accumulation
and the residual/LN stream are fp32.

Algebraic folds done on the host:
  M   = Wq @ Wk^T          -> scores = x M x^T  (one fused tensor A = x@M)
  NP  = Wv @ Wo            -> attn_out @ Wo = softmax(S) @ (x@NP) + bo2
  xpb = x + bo2            (residual-side bias pre-added on host)
  wrow = scale*(x @ (Wk@bq) + bq.bk) - SHIFT   (key-side score bias; the
        query-side bias and the constant SHIFT cancel in softmax)
  W1f = diag(g1) @ W1, bf1f = bf1 + b1 @ W1    (LN1 gamma/beta folded into
        FFN1 so the transposed LN1 output is spilled pre-gamma/beta)
  g1s/b1s/bf2s = 2^k-scaled LN1 spill constants matching W2's fp8 quant
        scale, so FFN2's dequant costs zero ops (LN2 is scale-invariant;
        eps is scaled to compensate exactly).

Weights are quantized to fp8 on the host with power-of-2 scales; dequant
factors fold into existing activation-scale immediates. x is uploaded as
bf16 (it only feeds fp8 matmuls) next to the f32 residual copy xpb.
Attention runs in transposed score layout S^T[k,q] and is software-
pipelined: scores of chunk qc+1 are emitted before the softmax tail of
chunk qc so TensorE never waits on the DVE row-sum tree. The residual+
LN1+h-transpose work lags one chunk further. FFN weights prefetch on the
sync queue during attention.

Self-contained: hardcodes shapes from the problem spec.
"""
import os

import numpy as np
import ml_dtypes

import concourse.bacc as bacc
import concourse.bass as bass
import concourse.tile as tile
import concourse.mybir as mybir
from concourse.bass_utils import run_bass_kernel_spmd
from concourse.masks import make_identity

P = 128
S = 2048          # sequence length per core
E = 1024          # embed
F = 4096          # ffn hidden
SB = S // P       # 16 seq blocks
EB = E // P       # 8 embed blocks
HB = F // P       # 32 ffn blocks
NCHUNK = 512
QC = S // NCHUNK  # 4 q chunks
QPC = NCHUNK // P  # 4 seq blocks per chunk
LN_EPS = 1e-5
SCALE = 1.0 / np.sqrt(np.float32(E))
EXP_SHIFT = 2.0   # subtracted inside exp for fp8 range headroom

F32 = mybir.dt.float32
BF16 = mybir.dt.bfloat16
FP8 = mybir.dt.float8e4
AF = mybir.ActivationFunctionType
ALU = mybir.AluOpType
DR = mybir.MatmulPerfMode.DoubleRow

_CACHED_NC = None


def _bcast_ap(ap, parts=P):
    """DRAM row-vector -> [parts, n] partition-broadcast access pattern."""
    return bass.AP(tensor=ap.tensor, offset=ap.offset,
                   ap=[[0, parts]] + [list(d) for d in ap.ap])


def build_nc(inv_sm, inv_snp, inv_sw1, eps2):
    nc = bacc.Bacc(None, target_bir_lowering=False, debug=False)

    # host-pretransposed+quantized x: xTq[p, eb*S + s] = x[s, eb*128+p]
    xt_d = nc.dram_tensor("xTq", [P, EB * S], FP8, kind="ExternalInput")
    xpb_d = nc.dram_tensor("xpb", [S, E], F32, kind="ExternalInput")
    # host-preshuffled: row p holds M[o*128+p, :] for o in 0..7, concatenated
    m_d = nc.dram_tensor("Mq", [P, EB * E], FP8, kind="ExternalInput")
    np_d = nc.dram_tensor("NPq", [P, EB * E], FP8, kind="ExternalInput")
    # W1q[c, p, t*E + ei*128 + j] = W1f[ei*128+p, (4c+t)*128+j]
    w1_d = nc.dram_tensor("W1q", [HB // 4, P, 4 * E], FP8, kind="ExternalInput")
    # W2q[p, hb*E + n] = W2[hb*128+p, n]
    w2_d = nc.dram_tensor("W2q", [P, HB * E], FP8, kind="ExternalInput")
    wrow_d = nc.dram_tensor("wrow", [S], F32, kind="ExternalInput")
    bf1_d = nc.dram_tensor("bf1f", [F], F32, kind="ExternalInput")
    g1_d = nc.dram_tensor("g1s", [E], F32, kind="ExternalInput")
    b1_d = nc.dram_tensor("b1s", [E], F32, kind="ExternalInput")
    g2_d = nc.dram_tensor("g2", [E], F32, kind="ExternalInput")
    b2_d = nc.dram_tensor("b2", [E], F32, kind="ExternalInput")
    out_d = nc.dram_tensor("out", [S, E], F32, kind="ExternalOutput")
    hs_d = nc.dram_tensor("hs_scratch", [S, E], F32)    # 2^k-scaled LN1 spill
    hT_d = nc.dram_tensor("hT_scratch", [E, S], FP8)    # transposed LN1 out

    with tile.TileContext(nc, pool_alloc_mode="queue") as tc:
        with tc.tile_pool(name="const", bufs=1) as const:
            ident = const.tile([P, P], BF16)
            make_identity(nc, ident)
            ones_c = const.tile([P, 1], F32)
            nc.vector.memset(ones_c[:], 1.0)
            eps_c = const.tile([P, 1], F32)
            nc.vector.memset(eps_c[:], LN_EPS)
            eps2_c = const.tile([P, 1], F32)
            nc.vector.memset(eps2_c[:], eps2)
            bf1_sb = const.tile([P, HB], F32)
            nc.sync.dma_start(bf1_sb[:], bf1_d[:].rearrange("(o p) -> p o", p=P))
            recip_sb = const.tile([P, SB], F32)
            w_sb = const.tile([P, SB], F32)
            nc.sync.dma_start(w_sb[:], wrow_d[:].rearrange("(o p) -> p o", p=P))

            # FFN weights/consts reserved early; DMAs emitted after Phase B
            # below so the x/M/NP loads win the sync queue first.
            pfE_cm = tc.tile_pool(name="pfE", bufs=1)
            pfE = pfE_cm.__enter__()
            w1_all = pfE.tile([P, HB // 4, 4, EB, P], FP8)
            w2_sb = pfE.tile([P, HB, E], FP8)

            with tc.tile_pool(name="pbig", bufs=1) as pbig:
                xT = pbig.tile([P, EB, S], FP8)  # xT[p,eb,s] = x[s, eb*P+p]
                nc.sync.dma_start(xT[:], xt_d[:].rearrange(
                    "p (o s) -> p o s", s=S))

                with tc.tile_pool(name="pkv", bufs=1) as pkv:
                    AT = pkv.tile([P, EB, S], FP8)   # (x@M)^T, unscaled
                    VW = pkv.tile([P, SB, E], FP8)   # x@NP, [k, f], unscaled

                    # ---- Phase B: AT, VW (fp8 DoubleRow) ------------------
                    with tc.tile_pool(name="wm", bufs=1) as wm, \
                         tc.tile_pool(name="pb_ps", bufs=4, space="PSUM") as pb_ps:
                        m_sb = wm.tile([P, EB, E], FP8)
                        np_sb = wm.tile([P, EB, E], FP8)
                        nc.sync.dma_start(m_sb[:], m_d[:].rearrange(
                            "p (o n) -> p o n", n=E))
                        nc.sync.dma_start(np_sb[:], np_d[:].rearrange(
                            "p (o n) -> p o n", n=E))
                        for eb in range(EB):
                            for qc in range(QC):
                                ps = pb_ps.tile([P, NCHUNK], F32, tag="at")
                                for i in range(EB // 2):
                                    nc.tensor.matmul(
                                        ps[:],
                                        m_sb[:, 2 * i:2 * i + 2,
                                             eb * P:(eb + 1) * P],
                                        xT[:, 2 * i:2 * i + 2,
                                           qc * NCHUNK:(qc + 1) * NCHUNK],
                                        start=(i == 0), stop=(i == EB // 2 - 1),
                                        perf_mode=DR)
                                nc.scalar.activation(
                                    AT[:, eb, qc * NCHUNK:(qc + 1) * NCHUNK],
                                    ps[:], AF.Copy, scale=float(inv_sm))
                        for sb in range(SB):
                            for ec in range(E // NCHUNK):
                                ps = pb_ps.tile([P, NCHUNK], F32, tag="vw")
                                for i in range(EB // 2):
                                    nc.tensor.matmul(
                                        ps[:],
                                        xT[:, 2 * i:2 * i + 2,
                                           sb * P:(sb + 1) * P],
                                        np_sb[:, 2 * i:2 * i + 2,
                                              ec * NCHUNK:(ec + 1) * NCHUNK],
                                        start=(i == 0), stop=(i == EB // 2 - 1),
                                        perf_mode=DR)
                                nc.vector.tensor_scalar(
                                    VW[:, sb, ec * NCHUNK:(ec + 1) * NCHUNK],
                                    ps[:], float(inv_snp), None, ALU.mult)
                        # FFN prefetch on the now-idle sync queue
                        for c in range(HB // 4):
                            nc.sync.dma_start(
                                w1_all[:, c], w1_d[c].rearrange(
                                    "p (t o n) -> p t o n", t=4, o=EB))
                        w2_r = w2_d[:].rearrange("p (o n) -> p o n", n=E)
                        for hq in range(4):
                            nc.sync.dma_start(
                                w2_sb[:, hq * (HB // 4):(hq + 1) * (HB // 4), :],
                                w2_r[:, hq * (HB // 4):(hq + 1) * (HB // 4), :])
                    # ---- Phase C: attention + proj, LN1 interleaved -------
                    with tc.tile_pool(name="pexp", bufs=2) as pexp, \
                         tc.tile_pool(name="pcw", bufs=2) as pcw, \
                         tc.tile_pool(name="pproj", bufs=5) as pproj, \
                         tc.tile_pool(name="lnc", bufs=1) as lnc, \
                         tc.tile_pool(name="pdw", bufs=2) as pdw, \
                     tc.tile_pool(name="pdx", bufs=2) as pdx, \
                         tc.tile_pool(name="pc_ps", bufs=3, space="PSUM") as pc_ps, \
                         tc.tile_pool(name="pp_ps", bufs=3, space="PSUM") as pp_ps, \
                         tc.tile_pool(name="pr_ps", bufs=1, space="PSUM") as pr_ps, \
                         tc.tile_pool(name="pdt_ps", bufs=1, space="PSUM") as pdt_ps:
                        g1_b = lnc.tile([P, E], F32)
                        b1_b = lnc.tile([P, E], F32)
                        nc.sync.dma_start(g1_b[:], _bcast_ap(g1_d[:]))
                        nc.sync.dma_start(b1_b[:], _bcast_ap(b1_d[:]))
                        hT_r = hT_d[:].rearrange("(o p) s -> p o s", p=P)
                        proj_tiles = {}

                        def d_chain(sb):
                            """residual + LN1 + transpose for one seq block."""
                            xf = pdw.tile([P, E], F32, tag="xres")
                            nc.sync.dma_start(xf[:], xpb_d[sb * P:(sb + 1) * P, :])
                            hpre = pdw.tile([P, E], F32, tag="hpre")
                            nc.vector.tensor_add(hpre[:], proj_tiles.pop(sb)[:],
                                                 xf[:])
                            stats = pdx.tile([P, 2, 6], F32, tag="ln_stats")
                            nc.vector.bn_stats(stats[:, 0, :], hpre[:, 0:512])
                            nc.vector.bn_stats(stats[:, 1, :], hpre[:, 512:1024])
                            mv = pdx.tile([P, 2], F32, tag="ln_mv")
                            nc.vector.bn_aggr(mv[:], stats[:])
                            std = pdx.tile([P, 1], F32, tag="ln_std")
                            nc.scalar.activation(std[:], mv[:, 1:2], AF.Sqrt,
                                                 bias=eps_c[:], scale=1.0)
                            rstd = pdx.tile([P, 1], F32, tag="ln_rstd")
                            nc.vector.reciprocal(rstd[:], std[:])
                            # n = (hpre - mu) * rstd  (normalized, pre-gamma)
                            nc.vector.tensor_scalar(hpre[:], hpre[:],
                                                    mv[:, 0:1], rstd[:],
                                                    ALU.subtract, ALU.mult)
                            hb16 = pdx.tile([P, E], BF16, tag="hb16")
                            nc.gpsimd.tensor_copy(hb16[:], hpre[:])
                            hTt = pdx.tile([P, EB, P], FP8, tag="hTt")
                            for t in range(2):
                                pt = pdt_ps.tile([P, 4, P], BF16, tag="tp2")
                                for k in range(4):
                                    eb = 4 * t + k
                                    nc.tensor.transpose(
                                        pt[:, k, :],
                                        hb16[:, eb * P:(eb + 1) * P], ident[:])
                                nc.scalar.copy(hTt[:, 4 * t:4 * t + 4, :],
                                               pt[:])
                            nc.scalar.dma_start(
                                hT_r[:, :, sb * P:(sb + 1) * P], hTt[:])
                            # scaled spill in place: hs = n*g1s + b1s
                            nc.gpsimd.tensor_mul(hpre[:], hpre[:], g1_b[:])
                            nc.gpsimd.tensor_add(hpre[:], hpre[:], b1_b[:])
                            nc.sync.dma_start(hs_d[sb * P:(sb + 1) * P, :],
                                              hpre[:])

                        def softmax_tail(qc, expS, acc):
                            """tree row-sums, recip, PV + proj for chunk qc."""
                            for j in range(4):
                                nc.vector.tensor_add(acc[j][:], acc[j][:],
                                                     acc[j + 4][:])
                            for j in range(2):
                                nc.vector.tensor_add(acc[j][:], acc[j][:],
                                                     acc[j + 2][:])
                            nc.vector.tensor_add(acc[0][:], acc[0][:], acc[1][:])
                            for qs in range(QPC):
                                sb = qc * QPC + qs
                                pr = pr_ps.tile([P, 1], F32, tag="rs")
                                nc.tensor.matmul(pr[:],
                                                 acc[0][:, qs * P:(qs + 1) * P],
                                                 ones_c[:], start=True, stop=True)
                                nc.vector.reciprocal(recip_sb[:, sb:sb + 1], pr[:])
                            for qs in range(QPC):
                                sb = qc * QPC + qs
                                proj = pproj.tile([P, E], BF16, tag="proj")
                                proj_tiles[sb] = proj
                                for fc in range(E // NCHUNK):
                                    ps = pp_ps.tile([P, NCHUNK], F32, tag="pp")
                                    for j in range(SB // 2):
                                        nc.tensor.matmul(
                                            ps[:],
                                            expS[:, 2 * j:2 * j + 2,
                                                 qs * P:(qs + 1) * P],
                                            VW[:, 2 * j:2 * j + 2,
                                               fc * NCHUNK:(fc + 1) * NCHUNK],
                                            start=(j == 0),
                                            stop=(j == SB // 2 - 1),
                                            perf_mode=DR)
                                    nc.scalar.activation(
                                        proj[:, fc * NCHUNK:(fc + 1) * NCHUNK],
                                        ps[:], AF.Copy,
                                        scale=recip_sb[:, sb:sb + 1])

                        expS_prev = None
                        acc_prev = None
                        for qc in range(QC):
                            expS = pexp.tile([P, SB, NCHUNK], FP8, tag="expS")
                            acc = [None] * 8
                            for kb in range(SB):
                                ps = pc_ps.tile([P, NCHUNK], F32, tag="s")
                                for i in range(EB // 2):
                                    nc.tensor.matmul(
                                        ps[:],
                                        xT[:, 2 * i:2 * i + 2,
                                           kb * P:(kb + 1) * P],
                                        AT[:, 2 * i:2 * i + 2,
                                           qc * NCHUNK:(qc + 1) * NCHUNK],
                                        start=(i == 0), stop=(i == EB // 2 - 1),
                                        perf_mode=DR)
                                nc.scalar.activation(
                                    expS[:, kb, :], ps[:], AF.Exp,
                                    bias=w_sb[:, kb:kb + 1], scale=float(SCALE))
                                if kb >= 8:
                                    j = kb - 8
                                    a = pcw.tile([P, NCHUNK], F32,
                                                 tag=f"acc{j}")
                                    nc.vector.tensor_add(a[:], expS[:, j, :],
                                                         expS[:, kb, :])
                                    acc[j] = a
                            if qc > 0:
                                softmax_tail(qc - 1, expS_prev, acc_prev)
                                for qs in range(QPC):
                                    if qc > 1:
                                        d_chain((qc - 2) * QPC + qs)
                            expS_prev = expS
                            acc_prev = acc
                        softmax_tail(QC - 1, expS_prev, acc_prev)
                        for qs in range(QPC):
                            d_chain((QC - 2) * QPC + qs)
                        for qs in range(QPC):
                            d_chain((QC - 1) * QPC + qs)
                # pkv, pbig closed

            # ---- Phase E: FFN + LN2 + out ---------------------------------
            with tc.tile_pool(name="pht", bufs=2) as pht, \
                 tc.tile_pool(name="lnc2", bufs=1) as lnc2, \
                 tc.tile_pool(name="pr1a", bufs=1) as pr1a, \
                 tc.tile_pool(name="pew", bufs=3) as pew, \
                 tc.tile_pool(name="pr1_ps", bufs=3, space="PSUM") as pr1_ps, \
                 tc.tile_pool(name="pf2_ps", bufs=4, space="PSUM") as pf2_ps:
                g2_b = lnc2.tile([P, E], F32)
                b2_b = lnc2.tile([P, E], F32)
                nc.sync.dma_start(g2_b[:], _bcast_ap(g2_d[:]))
                nc.sync.dma_start(b2_b[:], _bcast_ap(b2_d[:]))
                hT_r = hT_d[:].rearrange("(o p) s -> p o s", p=P)
                QW = 4 * P  # 4 seq blocks per group
                for g in range(S // QW):
                    hts = pht.tile([P, EB, QW], FP8, tag="hts")
                    nc.scalar.dma_start(hts[:], hT_r[:, :, g * QW:(g + 1) * QW])
                    r1_all = pr1a.tile([P, HB, QW], FP8, tag="r1a")
                    for c in range(HB // 4):
                        for t in range(4):
                            hb = c * 4 + t
                            ps1 = pr1_ps.tile([P, QW], F32, tag="r1")
                            for i in range(EB // 2):
                                nc.tensor.matmul(
                                    ps1[:],
                                    w1_all[:, c, t, 2 * i:2 * i + 2, :],
                                    hts[:, 2 * i:2 * i + 2, :],
                                    start=(i == 0), stop=(i == EB // 2 - 1),
                                    perf_mode=DR)
                            nc.scalar.activation(r1_all[:, hb, :], ps1[:],
                                                 AF.Relu,
                                                 bias=bf1_sb[:, hb:hb + 1],
                                                 scale=float(inv_sw1))
                    for i in range(QW // P):
                        sb = g * (QW // P) + i
                        t = pew.tile([P, E], F32, tag="ffn")
                        hres = pew.tile([P, E], F32, tag="hres")
                        nc.sync.dma_start(hres[:], hs_d[sb * P:(sb + 1) * P, :])
                        for j in range(E // NCHUNK):
                            ps = pf2_ps.tile([P, NCHUNK], F32, tag="f2")
                            for p2 in range(HB // 2):
                                nc.tensor.matmul(
                                    ps[:],
                                    r1_all[:, 2 * p2:2 * p2 + 2,
                                           i * P:(i + 1) * P],
                                    w2_sb[:, 2 * p2:2 * p2 + 2,
                                          j * NCHUNK:(j + 1) * NCHUNK],
                                    start=(p2 == 0), stop=(p2 == HB // 2 - 1),
                                    perf_mode=DR)
                            # hres carries (b1+bf2)*2^k from the spill fold
                            nc.vector.tensor_add(
                                t[:, j * NCHUNK:(j + 1) * NCHUNK], ps[:],
                                hres[:, j * NCHUNK:(j + 1) * NCHUNK])
                        # LN2 on 2^k-scaled stream (eps2 = eps * 2^2k)
                        stats = pew.tile([P, 2, 6], F32, tag="ln2_stats")
                        nc.vector.bn_stats(stats[:, 0, :], t[:, 0:512])
                        nc.vector.bn_stats(stats[:, 1, :], t[:, 512:1024])
                        mv = pew.tile([P, 2], F32, tag="ln2_mv")
                        nc.vector.bn_aggr(mv[:], stats[:])
                        std = pew.tile([P, 1], F32, tag="ln2_std")
                        nc.scalar.activation(std[:], mv[:, 1:2], AF.Sqrt,
                                             bias=eps2_c[:], scale=1.0)
                        rstd = pew.tile([P, 1], F32, tag="ln2_rstd")
                        nc.vector.reciprocal(rstd[:], std[:])
                        nc.vector.tensor_scalar(t[:], t[:], mv[:, 0:1], rstd[:],
                                                ALU.subtract, ALU.mult)
                        o = pew.tile([P, E], F32, tag="outt")
                        nc.vector.scalar_tensor_tensor(
                            o[:], t[:], 1.0, g2_b[:], ALU.mult, ALU.mult)
                        nc.vector.tensor_add(o[:], o[:], b2_b[:])
                        nc.sync.dma_start(out_d[sb * P:(sb + 1) * P, :], o[:])
            pfE_cm.__exit__(None, None, None)

    nc.compile()
    return nc


def _pow2_scale(absmax):
    """Largest power of two s with absmax * s <= 224."""
    return float(2.0 ** np.floor(np.log2(224.0 / absmax)))


def kernel(**inputs):
    global _CACHED_NC
    x = np.ascontiguousarray(np.asarray(inputs["x"], dtype=np.float32))
    B = x.shape[0]
    assert x.shape == (8, S, E), x.shape

    def q8(a, scale):
        return np.ascontiguousarray(
            (np.asarray(a, np.float32) * np.float32(scale))
            .astype(ml_dtypes.float8_e4m3))

    def f32(a):
        return np.ascontiguousarray(np.asarray(a, dtype=np.float32))

    Wq = np.asarray(inputs["Wq"], np.float32)
    Wk = np.asarray(inputs["Wk"], np.float32)
    Wv = np.asarray(inputs["Wv"], np.float32)
    Wo = np.asarray(inputs["Wo"], np.float32)
    bq = np.asarray(inputs["bq"], np.float32)
    bk = np.asarray(inputs["bk"], np.float32)
    bv = np.asarray(inputs["bv"], np.float32)
    bo = np.asarray(inputs["bo"], np.float32)
    g1 = np.asarray(inputs["g1"], np.float32)
    b1 = np.asarray(inputs["b1"], np.float32)
    W1 = np.asarray(inputs["W1"], np.float32)
    W2 = np.asarray(inputs["W2"], np.float32)
    bf1 = np.asarray(inputs["bf1"], np.float32)
    bf2 = np.asarray(inputs["bf2"], np.float32)
    scale = np.float32(SCALE)

    M = Wq @ Wk.T
    NP_ = Wv @ Wo
    W1f = W1 * g1[:, None]
    bf1f = bf1 + b1 @ W1

    sm = _pow2_scale(np.abs(M).max())
    snp = _pow2_scale(np.abs(NP_).max())
    sw1 = _pow2_scale(np.abs(W1f).max())
    sw2 = _pow2_scale(np.abs(W2).max())

    # shuffles: row p of Mq holds M[o*128+p, :] blocks concatenated over o
    Mq = q8(M, sm).reshape(EB, P, E).transpose(1, 0, 2).reshape(P, EB * E)
    NPq = q8(NP_, snp).reshape(EB, P, E).transpose(1, 0, 2).reshape(P, EB * E)
    # W1q[c, p, t*E + ei*128 + j] = W1f[ei*128+p, (4c+t)*128+j]
    W1q = (q8(W1f, sw1).reshape(EB, P, HB // 4, 4, P)
           .transpose(2, 1, 3, 0, 4).reshape(HB // 4, P, 4 * E))
    W2q = q8(W2, sw2).reshape(HB, P, E).transpose(1, 0, 2).reshape(P, HB * E)

    bo2 = bo + bv @ Wo
    shared = {
        "Mq": np.ascontiguousarray(Mq), "NPq": np.ascontiguousarray(NPq),
        "W1q": np.ascontiguousarray(W1q), "W2q": np.ascontiguousarray(W2q),
        "bf1f": f32(bf1f),
        "g1s": f32(g1 * sw2), "b1s": f32((b1 + bf2) * sw2),
        "g2": f32(inputs["g2"]), "b2": f32(inputs["b2"]),
    }
    vq = Wk @ bq
    cq = float(bq @ bk)

    def xt_shuffle(xc):
        # xTq[p, eb*S + s] = x[s, eb*128+p]
        xq = q8(xc, 1.0)
        return np.ascontiguousarray(
            xq.T.reshape(EB, P, S).transpose(1, 0, 2).reshape(P, EB * S))

    in_maps = [
        {"xTq": xt_shuffle(x[c]),
         "xpb": f32(x[c] + bo2),
         "wrow": f32(scale * (x[c] @ vq) + scale * cq - EXP_SHIFT),
         **shared}
        for c in range(B)
    ]

    if _CACHED_NC is None:
        _CACHED_NC = build_nc(1.0 / sm, 1.0 / snp, 1.0 / sw1,
                              LN_EPS * sw2 * sw2)
    nc = _CACHED_NC
    trace = bool(int(os.environ.get("BERT_TRACE", "0")))
    res = run_bass_kernel_spmd(nc, in_maps, core_ids=list(range(B)), trace=trace)
    if trace and res.exec_time_ns is not None:
        print(f"HW exec time: {res.exec_time_ns} ns")
        kernel.last_exec_time_ns = res.exec_time_ns
        kernel.last_trace = res.instructions_and_trace
    return np.stack([res.results[c]["out"] for c in range(B)]).astype(np.float32)


# revision 34
# speedup vs baseline: 1.1186x; 1.1186x over previous
"""BERT encoder block on 8 Trainium2 NeuronCores.

Strategy: pure data parallelism — batch 8 is split one batch element per core
(no collectives). Each core runs the full encoder block on its [2048, 1024]
slice. All six large matmuls run in fp8e4m3 with DoubleRow perf mode
(K=256 contracted per instruction, ~1.7x TensorE throughput); accumulation
and the residual/LN stream are fp32.

Algebraic folds done on the host:
  M   = Wq @ Wk^T          -> scores = x M x^T  (one fused tensor A = x@M)
  NP  = Wv @ Wo            -> attn_out @ Wo = softmax(S) @ (x@NP) + bo2
  xpb = x + bo2            (residual-side bias pre-added on host)
  wrow = scale*(x @ (Wk@bq) + bq.bk) - SHIFT   (key-side score bias; the
        query-side bias and the constant SHIFT cancel in softmax)
  W1f = diag(g1) @ W1, bf1f = bf1 + b1 @ W1    (LN1 gamma/beta folded into
        FFN1 so the transposed LN1 output is spilled pre-gamma/beta)
  g1s/b1s/bf2s = 2^k-scaled LN1 spill constants matching W2's fp8 quant
        scale, so FFN2's dequant costs zero ops (LN2 is scale-invariant;
        eps is scaled to compensate exactly).

Weights are quantized to fp8 on the host with power-of-2 scales; dequant
factors fold into existing activation-scale immediates. x is uploaded as
bf16 (it only feeds fp8 matmuls) next to the f32 residual copy xpb.
Attention runs in transposed score layout S^T[k,q] and is software-
pipelined: scores of chunk qc+1 are emitted before the softmax tail of
chunk qc so TensorE never waits on the DVE row-sum tree. The residual+
LN1+h-transpose work lags one chunk further. FFN weights prefetch on the
sync queue during attention.

Self-contained: hardcodes shapes from the problem spec.
"""
import os

import numpy as np
import ml_dtypes

import concourse.bacc as bacc  # noqa: F401
import concourse.bass as bass
import concourse.tile as tile
import concourse.mybir as mybir
from concourse.bass_utils import run_bass_kernel_spmd
from concourse.masks import make_identity

P = 128
S = 2048          # sequence length per core
E = 1024          # embed
F = 4096          # ffn hidden
SB = S // P       # 16 seq blocks
EB = E // P       # 8 embed blocks
HB = F // P       # 32 ffn blocks
NCHUNK = 512
QC = S // NCHUNK  # 4 q chunks
QPC = NCHUNK // P  # 4 seq blocks per chunk
LN_EPS = 1e-5
SCALE = 1.0 / np.sqrt(np.float32(E))
EXP_SHIFT = 2.0   # subtracted inside exp for fp8 range headroom

F32 = mybir.dt.float32
BF16 = mybir.dt.bfloat16
FP8 = mybir.dt.float8e4
AF = mybir.ActivationFunctionType
ALU = mybir.AluOpType
DR = mybir.MatmulPerfMode.DoubleRow

_CACHED_NC = None


def _bcast_ap(ap, parts=P):
    """DRAM row-vector -> [parts, n] partition-broadcast access pattern."""
    return bass.AP(tensor=ap.tensor, offset=ap.offset,
                   ap=[[0, parts]] + [list(d) for d in ap.ap])


def build_nc(inv_sm, inv_snp, inv_sw1, eps2):
    nc = bacc.Bacc(None, target_bir_lowering=False, debug=False)

    # host-pretransposed+quantized x: xTq[p, eb*S + s] = x[s, eb*128+p]
    xt_d = nc.dram_tensor("xTq", [P, EB * S], FP8, kind="ExternalInput")
    xpb_d = nc.dram_tensor("xpb", [S, E], F32, kind="ExternalInput")
    # host-preshuffled: row p holds M[o*128+p, :] for o in 0..7, concatenated
    m_d = nc.dram_tensor("Mq", [P, EB * E], FP8, kind="ExternalInput")
    np_d = nc.dram_tensor("NPq", [P, EB * E], FP8, kind="ExternalInput")
    # W1q[c, p, t*E + ei*128 + j] = W1f[ei*128+p, (4c+t)*128+j]
    w1_d = nc.dram_tensor("W1q", [HB // 4, P, 4 * E], FP8, kind="ExternalInput")
    # W2q[p, hb*E + n] = W2[hb*128+p, n]
    w2_d = nc.dram_tensor("W2q", [P, HB * E], FP8, kind="ExternalInput")
    wrow_d = nc.dram_tensor("wrow", [S], F32, kind="ExternalInput")
    bf1_d = nc.dram_tensor("bf1f", [F], F32, kind="ExternalInput")
    g1_d = nc.dram_tensor("g1s", [E], F32, kind="ExternalInput")
    b1_d = nc.dram_tensor("b1s", [E], F32, kind="ExternalInput")
    g2_d = nc.dram_tensor("g2", [E], F32, kind="ExternalInput")
    b2_d = nc.dram_tensor("b2", [E], F32, kind="ExternalInput")
    out_d = nc.dram_tensor("out", [S, E], F32, kind="ExternalOutput")
    hs_d = nc.dram_tensor("hs_scratch", [S, E], F32)    # 2^k-scaled LN1 spill
    hT_d = nc.dram_tensor("hT_scratch", [E, S], FP8)    # transposed LN1 out

    with tile.TileContext(nc, pool_alloc_mode="queue") as tc:
        with tc.tile_pool(name="const", bufs=1) as const:
            ident = const.tile([P, P], BF16)
            make_identity(nc, ident)
            ones_c = const.tile([P, 1], F32)
            nc.vector.memset(ones_c[:], 1.0)
            eps_c = const.tile([P, 1], F32)
            nc.vector.memset(eps_c[:], LN_EPS)
            eps2_c = const.tile([P, 1], F32)
            nc.vector.memset(eps2_c[:], eps2)
            bf1_sb = const.tile([P, HB], F32)
            nc.sync.dma_start(bf1_sb[:], bf1_d[:].rearrange("(o p) -> p o", p=P))
            recip_sb = const.tile([P, SB], F32)
            w_sb = const.tile([P, SB], F32)
            nc.sync.dma_start(w_sb[:], wrow_d[:].rearrange("(o p) -> p o", p=P))

            # FFN weights/consts reserved early; DMAs emitted after Phase B
            # below so the x/M/NP loads win the sync queue first.
            pfE_cm = tc.tile_pool(name="pfE", bufs=1)
            pfE = pfE_cm.__enter__()
            w1_all = pfE.tile([P, HB // 4, 4, EB, P], FP8)
            w2_sb = pfE.tile([P, HB, E], FP8)

            with tc.tile_pool(name="pbig", bufs=1) as pbig:
                xT = pbig.tile([P, EB, S], FP8)  # xT[p,eb,s] = x[s, eb*P+p]
                xt_r = xt_d[:].rearrange("p (o s) -> p o s", s=S)
                for qc in range(QC):
                    nc.sync.dma_start(
                        xT[:, :, qc * NCHUNK:(qc + 1) * NCHUNK],
                        xt_r[:, :, qc * NCHUNK:(qc + 1) * NCHUNK])

                with tc.tile_pool(name="pkv", bufs=1) as pkv:
                    AT = pkv.tile([P, EB, S], FP8)   # (x@M)^T, unscaled
                    VW = pkv.tile([P, SB, E], FP8)   # x@NP, [k, f], unscaled

                    # ---- Phase B: AT, VW (fp8 DoubleRow) ------------------
                    with tc.tile_pool(name="wm", bufs=1) as wm, \
                         tc.tile_pool(name="pb_ps", bufs=4, space="PSUM") as pb_ps:
                        m_sb = wm.tile([P, EB, E], FP8)
                        np_sb = wm.tile([P, EB, E], FP8)
                        nc.scalar.dma_start(m_sb[:], m_d[:].rearrange(
                            "p (o n) -> p o n", n=E))
                        nc.scalar.dma_start(np_sb[:], np_d[:].rearrange(
                            "p (o n) -> p o n", n=E))
                        for eb in range(EB):
                            for qc in range(QC):
                                ps = pb_ps.tile([P, NCHUNK], F32, tag="at")
                                for i in range(EB // 2):
                                    nc.tensor.matmul(
                                        ps[:],
                                        m_sb[:, 2 * i:2 * i + 2,
                                             eb * P:(eb + 1) * P],
                                        xT[:, 2 * i:2 * i + 2,
                                           qc * NCHUNK:(qc + 1) * NCHUNK],
                                        start=(i == 0), stop=(i == EB // 2 - 1),
                                        perf_mode=DR)
                                nc.scalar.activation(
                                    AT[:, eb, qc * NCHUNK:(qc + 1) * NCHUNK],
                                    ps[:], AF.Copy, scale=float(inv_sm))
                        for sb in range(SB):
                            for ec in range(E // NCHUNK):
                                ps = pb_ps.tile([P, NCHUNK], F32, tag="vw")
                                for i in range(EB // 2):
                                    nc.tensor.matmul(
                                        ps[:],
                                        xT[:, 2 * i:2 * i + 2,
                                           sb * P:(sb + 1) * P],
                                        np_sb[:, 2 * i:2 * i + 2,
                                              ec * NCHUNK:(ec + 1) * NCHUNK],
                                        start=(i == 0), stop=(i == EB // 2 - 1),
                                        perf_mode=DR)
                                nc.vector.tensor_scalar(
                                    VW[:, sb, ec * NCHUNK:(ec + 1) * NCHUNK],
                                    ps[:], float(inv_snp), None, ALU.mult)
                        # FFN prefetch on the now-idle sync queue
                        for c in range(HB // 4):
                            nc.sync.dma_start(
                                w1_all[:, c], w1_d[c].rearrange(
                                    "p (t o n) -> p t o n", t=4, o=EB))
                        w2_r = w2_d[:].rearrange("p (o n) -> p o n", n=E)
                        for hq in range(4):
                            nc.sync.dma_start(
                                w2_sb[:, hq * (HB // 4):(hq + 1) * (HB // 4), :],
                                w2_r[:, hq * (HB // 4):(hq + 1) * (HB // 4), :])
                    # ---- Phase C: attention + proj, LN1 interleaved -------
                    with tc.tile_pool(name="pexp", bufs=2) as pexp, \
                         tc.tile_pool(name="pcw", bufs=1) as pcw, \
                         tc.tile_pool(name="pproj", bufs=5) as pproj, \
                         tc.tile_pool(name="lnc", bufs=1) as lnc, \
                         tc.tile_pool(name="pdw", bufs=2) as pdw, \
                     tc.tile_pool(name="pdx", bufs=2) as pdx, \
                         tc.tile_pool(name="pc_ps", bufs=3, space="PSUM") as pc_ps, \
                         tc.tile_pool(name="pp_ps", bufs=2, space="PSUM") as pp_ps, \
                         tc.tile_pool(name="pr_ps", bufs=1, space="PSUM") as pr_ps, \
                         tc.tile_pool(name="pdt_ps", bufs=2, space="PSUM") as pdt_ps:
                        g1_b = lnc.tile([P, E], F32)
                        b1_b = lnc.tile([P, E], F32)
                        nc.sync.dma_start(g1_b[:], _bcast_ap(g1_d[:]))
                        nc.sync.dma_start(b1_b[:], _bcast_ap(b1_d[:]))
                        hT_r = hT_d[:].rearrange("(o p) s -> p o s", p=P)
                        proj_tiles = {}

                        def d_chain(sb):
                            """residual + LN1 + transpose for one seq block."""
                            xf = pdw.tile([P, E], F32, tag="xres")
                            nc.sync.dma_start(xf[:], xpb_d[sb * P:(sb + 1) * P, :])
                            hpre = pdw.tile([P, E], F32, tag="hpre")
                            nc.vector.tensor_add(hpre[:], proj_tiles.pop(sb)[:],
                                                 xf[:])
                            stats = pdx.tile([P, 2, 6], F32, tag="ln_stats")
                            nc.vector.bn_stats(stats[:, 0, :], hpre[:, 0:512])
                            nc.vector.bn_stats(stats[:, 1, :], hpre[:, 512:1024])
                            mv = pdx.tile([P, 2], F32, tag="ln_mv")
                            nc.vector.bn_aggr(mv[:], stats[:])
                            std = pdx.tile([P, 1], F32, tag="ln_std")
                            nc.scalar.activation(std[:], mv[:, 1:2], AF.Sqrt,
                                                 bias=eps_c[:], scale=1.0)
                            rstd = pdx.tile([P, 1], F32, tag="ln_rstd")
                            nc.vector.reciprocal(rstd[:], std[:])
                            # n = (hpre - mu) * rstd  (normalized, pre-gamma)
                            nc.vector.tensor_scalar(hpre[:], hpre[:],
                                                    mv[:, 0:1], rstd[:],
                                                    ALU.subtract, ALU.mult)
                            hb16 = pdx.tile([P, E], BF16, tag="hb16")
                            nc.vector.tensor_copy(hb16[:], hpre[:])
                            hTt = pdx.tile([P, EB, P], FP8, tag="hTt")
                            for t in range(2):
                                pt = pdt_ps.tile([P, 4, P], BF16, tag="tp2")
                                for k in range(4):
                                    eb = 4 * t + k
                                    nc.tensor.transpose(
                                        pt[:, k, :],
                                        hb16[:, eb * P:(eb + 1) * P], ident[:])
                                nc.scalar.copy(hTt[:, 4 * t:4 * t + 4, :],
                                               pt[:])
                            nc.scalar.dma_start(
                                hT_r[:, :, sb * P:(sb + 1) * P], hTt[:])
                            # scaled spill in place: hs = n*g1s + b1s
                            nc.gpsimd.tensor_mul(hpre[:], hpre[:], g1_b[:])
                            nc.gpsimd.tensor_add(hpre[:], hpre[:], b1_b[:])
                            nc.sync.dma_start(hs_d[sb * P:(sb + 1) * P, :],
                                              hpre[:])

                        def softmax_tail(qc, expS):
                            """tree row-sums, recip, PV + proj for chunk qc."""
                            acc = [None] * 8
                            for j in range(8):
                                a = pcw.tile([P, NCHUNK], F32, tag=f"acc{j}")
                                nc.vector.tensor_add(a[:], expS[:, j, :],
                                                     expS[:, j + 8, :])
                                acc[j] = a
                            for j in range(4):
                                nc.vector.tensor_add(acc[j][:], acc[j][:],
                                                     acc[j + 4][:])
                            for j in range(2):
                                nc.vector.tensor_add(acc[j][:], acc[j][:],
                                                     acc[j + 2][:])
                            nc.vector.tensor_add(acc[0][:], acc[0][:], acc[1][:])
                            for qs in range(QPC):
                                sb = qc * QPC + qs
                                pr = pr_ps.tile([P, 1], F32, tag="rs")
                                nc.tensor.matmul(pr[:],
                                                 acc[0][:, qs * P:(qs + 1) * P],
                                                 ones_c[:], start=True, stop=True)
                                nc.vector.reciprocal(recip_sb[:, sb:sb + 1], pr[:])
                            for qs in range(QPC):
                                sb = qc * QPC + qs
                                proj = pproj.tile([P, E], BF16, tag="proj")
                                proj_tiles[sb] = proj
                                for fc in range(E // NCHUNK):
                                    ps = pp_ps.tile([P, NCHUNK], F32, tag="pp")
                                    for j in range(SB // 2):
                                        nc.tensor.matmul(
                                            ps[:],
                                            expS[:, 2 * j:2 * j + 2,
                                                 qs * P:(qs + 1) * P],
                                            VW[:, 2 * j:2 * j + 2,
                                               fc * NCHUNK:(fc + 1) * NCHUNK],
                                            start=(j == 0),
                                            stop=(j == SB // 2 - 1),
                                            perf_mode=DR)
                                    nc.scalar.activation(
                                        proj[:, fc * NCHUNK:(fc + 1) * NCHUNK],
                                        ps[:], AF.Copy,
                                        scale=recip_sb[:, sb:sb + 1])

                        expS_prev = None
                        for qc in range(QC):
                            expS = pexp.tile([P, SB, NCHUNK], FP8, tag="expS")
                            for kb in range(SB):
                                ps = pc_ps.tile([P, NCHUNK], F32, tag="s")
                                for i in range(EB // 2):
                                    nc.tensor.matmul(
                                        ps[:],
                                        xT[:, 2 * i:2 * i + 2,
                                           kb * P:(kb + 1) * P],
                                        AT[:, 2 * i:2 * i + 2,
                                           qc * NCHUNK:(qc + 1) * NCHUNK],
                                        start=(i == 0), stop=(i == EB // 2 - 1),
                                        perf_mode=DR)
                                nc.scalar.activation(
                                    expS[:, kb, :], ps[:], AF.Exp,
                                    bias=w_sb[:, kb:kb + 1], scale=float(SCALE))
                            if qc > 0:
                                softmax_tail(qc - 1, expS_prev)
                                for qs in range(QPC):
                                    if qc > 1:
                                        d_chain((qc - 2) * QPC + qs)
                            expS_prev = expS
                        softmax_tail(QC - 1, expS_prev)
                        for qs in range(QPC):
                            d_chain((QC - 2) * QPC + qs)
                        for qs in range(QPC):
                            d_chain((QC - 1) * QPC + qs)
                # pkv, pbig closed

            # ---- Phase E: FFN + LN2 + out ---------------------------------
            with tc.tile_pool(name="pht", bufs=2) as pht, \
                 tc.tile_pool(name="lnc2", bufs=1) as lnc2, \
                 tc.tile_pool(name="pr1a", bufs=1) as pr1a, \
                 tc.tile_pool(name="pew", bufs=3) as pew, \
                 tc.tile_pool(name="pr1_ps", bufs=3, space="PSUM") as pr1_ps, \
                 tc.tile_pool(name="pf2_ps", bufs=4, space="PSUM") as pf2_ps:
                g2_b = lnc2.tile([P, E], F32)
                b2_b = lnc2.tile([P, E], F32)
                nc.sync.dma_start(g2_b[:], _bcast_ap(g2_d[:]))
                nc.sync.dma_start(b2_b[:], _bcast_ap(b2_d[:]))
                hT_r = hT_d[:].rearrange("(o p) s -> p o s", p=P)
                QW = 4 * P  # 4 seq blocks per group
                for g in range(S // QW):
                    hts = pht.tile([P, EB, QW], FP8, tag="hts")
                    nc.scalar.dma_start(hts[:], hT_r[:, :, g * QW:(g + 1) * QW])
                    r1_all = pr1a.tile([P, HB, QW], FP8, tag="r1a")
                    for c in range(HB // 4):
                        for t in range(4):
                            hb = c * 4 + t
                            ps1 = pr1_ps.tile([P, QW], F32, tag="r1")
                            for i in range(EB // 2):
                                nc.tensor.matmul(
                                    ps1[:],
                                    w1_all[:, c, t, 2 * i:2 * i + 2, :],
                                    hts[:, 2 * i:2 * i + 2, :],
                                    start=(i == 0), stop=(i == EB // 2 - 1),
                                    perf_mode=DR)
                            nc.scalar.activation(r1_all[:, hb, :], ps1[:],
                                                 AF.Relu,
                                                 bias=bf1_sb[:, hb:hb + 1],
                                                 scale=float(inv_sw1))
                    for i in range(QW // P):
                        sb = g * (QW // P) + i
                        t = pew.tile([P, E], F32, tag="ffn")
                        hres = pew.tile([P, E], F32, tag="hres")
                        nc.sync.dma_start(hres[:], hs_d[sb * P:(sb + 1) * P, :])
                        for j in range(E // NCHUNK):
                            ps = pf2_ps.tile([P, NCHUNK], F32, tag="f2")
                            for p2 in range(HB // 2):
                                nc.tensor.matmul(
                                    ps[:],
                                    r1_all[:, 2 * p2:2 * p2 + 2,
                                           i * P:(i + 1) * P],
                                    w2_sb[:, 2 * p2:2 * p2 + 2,
                                          j * NCHUNK:(j + 1) * NCHUNK],
                                    start=(p2 == 0), stop=(p2 == HB // 2 - 1),
                                    perf_mode=DR)
                            # hres carries (b1+bf2)*2^k from the spill fold
                            nc.vector.tensor_add(
                                t[:, j * NCHUNK:(j + 1) * NCHUNK], ps[:],
                                hres[:, j * NCHUNK:(j + 1) * NCHUNK])
                        # LN2 on 2^k-scaled stream (eps2 = eps * 2^2k)
                        stats = pew.tile([P, 2, 6], F32, tag="ln2_stats")
                        nc.vector.bn_stats(stats[:, 0, :], t[:, 0:512])
                        nc.vector.bn_stats(stats[:, 1, :], t[:, 512:1024])
                        mv = pew.tile([P, 2], F32, tag="ln2_mv")
                        nc.vector.bn_aggr(mv[:], stats[:])
                        std = pew.tile([P, 1], F32, tag="ln2_std")
                        nc.scalar.activation(std[:], mv[:, 1:2], AF.Sqrt,
                                             bias=eps2_c[:], scale=1.0)
                        rstd = pew.tile([P, 1], F32, tag="ln2_rstd")
                        nc.vector.reciprocal(rstd[:], std[:])
                        nc.vector.tensor_scalar(t[:], t[:], mv[:, 0:1], rstd[:],
                                                ALU.subtract, ALU.mult)
                        o = pew.tile([P, E], F32, tag="outt")
                        nc.vector.scalar_tensor_tensor(
                            o[:], t[:], 1.0, g2_b[:], ALU.mult, ALU.mult)
                        nc.vector.tensor_add(o[:], o[:], b2_b[:])
                        nc.sync.dma_start(out_d[sb * P:(sb + 1) * P, :], o[:])
            pfE_cm.__exit__(None, None, None)

    nc.compile()
    return nc


def _pow2_scale(absmax):
    """Largest power of two s with absmax * s <= 224."""
    return float(2.0 ** np.floor(np.log2(224.0 / absmax)))


def kernel(**inputs):
    global _CACHED_NC
    x = np.ascontiguousarray(np.asarray(inputs["x"], dtype=np.float32))
    B = x.shape[0]
    assert x.shape == (8, S, E), x.shape

    def q8(a, scale):
        return np.ascontiguousarray(
            (np.asarray(a, np.float32) * np.float32(scale))
            .astype(ml_dtypes.float8_e4m3))

    def f32(a):
        return np.ascontiguousarray(np.asarray(a, dtype=np.float32))

    Wq = np.asarray(inputs["Wq"], np.float32)
    Wk = np.asarray(inputs["Wk"], np.float32)
    Wv = np.asarray(inputs["Wv"], np.float32)
    Wo = np.asarray(inputs["Wo"], np.float32)
    bq = np.asarray(inputs["bq"], np.float32)
    bk = np.asarray(inputs["bk"], np.float32)
    bv = np.asarray(inputs["bv"], np.float32)
    bo = np.asarray(inputs["bo"], np.float32)
    g1 = np.asarray(inputs["g1"], np.float32)
    b1 = np.asarray(inputs["b1"], np.float32)
    W1 = np.asarray(inputs["W1"], np.float32)
    W2 = np.asarray(inputs["W2"], np.float32)
    bf1 = np.asarray(inputs["bf1"], np.float32)
    bf2 = np.asarray(inputs["bf2"], np.float32)
    scale = np.float32(SCALE)

    M = Wq @ Wk.T
    NP_ = Wv @ Wo
    W1f = W1 * g1[:, None]
    bf1f = bf1 + b1 @ W1

    sm = _pow2_scale(np.abs(M).max())
    snp = _pow2_scale(np.abs(NP_).max())
    sw1 = _pow2_scale(np.abs(W1f).max())
    sw2 = _pow2_scale(np.abs(W2).max())

    # shuffles: row p of Mq holds M[o*128+p, :] blocks concatenated over o
    Mq = q8(M, sm).reshape(EB, P, E).transpose(1, 0, 2).reshape(P, EB * E)
    NPq = q8(NP_, snp).reshape(EB, P, E).transpose(1, 0, 2).reshape(P, EB * E)
    # W1q[c, p, t*E + ei*128 + j] = W1f[ei*128+p, (4c+t)*128+j]
    W1q = (q8(W1f, sw1).reshape(EB, P, HB // 4, 4, P)
           .transpose(2, 1, 3, 0, 4).reshape(HB // 4, P, 4 * E))
    W2q = q8(W2, sw2).reshape(HB, P, E).transpose(1, 0, 2).reshape(P, HB * E)

    bo2 = bo + bv @ Wo
    shared = {
        "Mq": np.ascontiguousarray(Mq), "NPq": np.ascontiguousarray(NPq),
        "W1q": np.ascontiguousarray(W1q), "W2q": np.ascontiguousarray(W2q),
        "bf1f": f32(bf1f),
        "g1s": f32(g1 * sw2), "b1s": f32((b1 + bf2) * sw2),
        "g2": f32(inputs["g2"]), "b2": f32(inputs["b2"]),
    }
    vq = Wk @ bq
    cq = float(bq @ bk)

    def xt_shuffle(xc):
        # xTq[p, eb*S + s] = x[s, eb*128+p]
        xq = q8(xc, 1.0)
        return np.ascontiguousarray(
            xq.T.reshape(EB, P, S).transpose(1, 0, 2).reshape(P, EB * S))

    in_maps = [
        {"xTq": xt_shuffle(x[c]),
         "xpb": f32(x[c] + bo2),
         "wrow": f32(scale * (x[c] @ vq) + scale * cq - EXP_SHIFT),
         **shared}
        for c in range(B)
    ]

    if _CACHED_NC is None:
        _CACHED_NC = build_nc(1.0 / sm, 1.0 / snp, 1.0 / sw1,
                              LN_EPS * sw2 * sw2)
    nc = _CACHED_NC
    trace = bool(int(os.environ.get("BERT_TRACE", "0")))
    res = run_bass_kernel_spmd(nc, in_maps, core_ids=list(range(B)), trace=trace)
    if trace and res.exec_time_ns is not None:
        print(f"HW exec time: {res.exec_time_ns} ns")
        kernel.last_exec_time_ns = res.exec_time_ns
        kernel.last_trace = res.instructions_and_trace
    return np.stack([res.results[c]["out"] for c in range(B)]).astype(np.float32)
